# revision 6
# baseline (speedup 1.0000x reference)
"""NT-Xent loss kernel for Trainium2, 8 NeuronCores.

Strategy (row-sharded similarity matrix, v3):
  - Each core receives the full feature matrix cyclically rolled by c*1024
    rows, so every core runs the identical program: its 1024 rows are
    rolled-rows [0, 1024), its positive columns are [4096, 5120).
  - Column groups processed outermost (g=0..3), row tiles m=0..7 inner;
    group g+1's load/normalize/transpose overlaps group g's exp stream.
  - Normalization: squares on GPSIMD (DVE for the head group), segmented
    reduce on DVE, rsqrt via ACT ln/exp, scale to bf16 on DVE.
  - PE transposes into a PSUM tile borrowed from the matmul pool at the
    group boundary; DVE copies it out to zbT in SBUF.
  - exp(10*sim) mostly on ScalarE (activation, accum_out = row sums); six
    chunks are offloaded to the DVE via the Schraudolph bit trick:
    int32(sim*A10 + BS) bitcast to f32 is a ~2%-accurate exp(10*sim)
    whose row-sum error washes out (<1e-4 in the final loss).
  - Diagonal self-sim is exp(10) exactly (z normalized) -> subtract a
    constant instead of extracting it.  Positives are computed directly
    as a bf16 dot product of zbg (rows block) with zbg (positive block).
  - loss_row = ln(rowsum - e^10) - 10*pos; per-core [128, 8] tile is
    DMA'd out; the host sums partials and divides by N.
"""

import os

import numpy as np

N = 8192
D = 128
NCORES = 8
RPC = N // NCORES          # rows per core = 1024
G = 4                      # column groups
GCOLS = N // G             # 2048 columns per group
RT = RPC // 128            # row tiles per core = 8
ESC = 10.0                 # 1 / temperature
E10 = float(np.exp(10.0))  # diagonal exp value (z normalized -> sim_ii = 10)

# Schraudolph exp constants: bits(exp(10*s)) ~ int32(s*A10 + BS).
A10 = 10.0 * (2.0 ** 23) / float(np.log(2.0))   # 121022032
BS = float(np.float32(127 * 2 ** 23 - 482000))  # calibrated for zero mean err

# Chunks (g, m) computed on the DVE instead of ScalarE. g=0 must stay on
# ACT (diagonal runs through exact exp so the e^10 subtraction cancels).
DVE_CHUNKS = {(1, 4), (1, 6), (2, 4), (2, 6), (3, 4), (3, 6)}

_CACHE = {}
LAST_RESULTS = None


def _patch_act_tables():
    """Force Exp/Ln onto the combined natural_log_exp_and_others table set.

    The greedy table-load pass otherwise alternates between exp-only and
    ln-only sets (one ~2.7us table load per switch).  Stripping Exp/Ln from
    the competing sets leaves exactly one set that can serve them, so a
    single load covers the whole kernel.
    """
    if _CACHE.get("act_patched"):
        return
    import functools

    import concourse.bacc as bacc_mod
    import concourse.bass_interp as interp_mod
    import concourse.hw_specs as hw_specs
    import concourse.mybir as mybir

    AF = mybir.ActivationFunctionType
    orig = hw_specs.get_activation_tables

    @functools.cache
    def patched(arch):
        out = {}
        for name, funcs in orig(arch).items():
            if name != "natural_log_exp_and_others":
                funcs = funcs - {AF.Exp, AF.Ln}
            out[name] = funcs
        return out

    hw_specs.get_activation_tables = patched
    bacc_mod.get_activation_tables = patched
    interp_mod.get_activation_tables = patched
    _CACHE["act_patched"] = True


def _build():
    import concourse.mybir as mybir
    import concourse.tile as tile
    from concourse import bacc

    _patch_act_tables()

    f32 = mybir.dt.float32
    bf16 = mybir.dt.bfloat16
    i32 = mybir.dt.int32
    AX = mybir.AxisListType
    OP = mybir.AluOpType
    AF = mybir.ActivationFunctionType

    nc = bacc.Bacc(
        "TRN2",
        target_bir_lowering=False,
        debug=False,
        enable_asserts=False,
        num_devices=NCORES,
    )
    x = nc.dram_tensor("x", [N, D], f32, kind="ExternalInput").ap()
    ident_in = nc.dram_tensor("ident", [128, 128], f32, kind="ExternalInput").ap()
    out = nc.dram_tensor("loss_parts", [128, RT], f32, kind="ExternalOutput").ap()

    with tile.TileContext(nc) as tc:
        with (
            tc.tile_pool(name="const", bufs=1) as constp,
            tc.tile_pool(name="big", bufs=1) as bigp,
            tc.tile_pool(name="small", bufs=2) as smallp,
            tc.tile_pool(name="psum", bufs=2, space="PSUM") as psump,
        ):
            # Touch Ln+Exp first so the ACT table load starts immediately.
            warm = constp.tile([128, 1], f32, tag="warm")
            nc.vector.memset(warm[:], 1.0)
            nc.scalar.activation(warm[:], warm[:], AF.Ln)
            nc.scalar.activation(warm[:], warm[:], AF.Exp)

            eps2 = constp.tile([128, 1], f32, tag="eps2")
            nc.vector.memset(eps2[:], 1e-16)
            ident = constp.tile([128, 128], bf16, tag="ident")
            identf = constp.tile([128, 128], f32, tag="identf")
            nc.sync.dma_start(out=identf[:], in_=ident_in)
            nc.vector.tensor_copy(ident[:], identf[:])

            # Dedicated (non-rotating) tiles: lifetimes are simple and SBUF
            # is plentiful, so avoid pool-recycling hazards entirely.
            xg = [bigp.tile([128, GCOLS], f32, tag=f"xg{g}", name=f"xg{g}") for g in range(G)]
            sq = [bigp.tile([128, GCOLS], f32, tag=f"sq{g}", name=f"sq{g}") for g in range(G)]
            zbg = [bigp.tile([128, GCOLS], bf16, tag=f"zbg{g}", name=f"zbg{g}") for g in range(G)]
            zbT = [bigp.tile([128, GCOLS], bf16, tag=f"zbT{g}", name=f"zbT{g}") for g in range(G)]
            nsq = [bigp.tile([128, 16], f32, tag=f"nsq{g}", name=f"nsq{g}") for g in range(G)]
            rno = [bigp.tile([128, 16], f32, tag=f"rno{g}", name=f"rno{g}") for g in range(G)]
            # exp destinations (ACT chunks) and Schraudolph ints (DVE chunks)
            et = [bigp.tile([128, GCOLS], f32, tag=f"et{k}", name=f"et{k}") for k in range(2)]
            ei = [bigp.tile([128, GCOLS], i32, tag=f"ei{k}", name=f"ei{k}") for k in range(2)]

            racc = constp.tile([128, G * RT + 4], f32, tag="racc")   # ACT sums
            rdve = constp.tile([128, G * RT], f32, tag="rdve")       # DVE sums
            nc.vector.memset(rdve[:], 0.0)
            pos = constp.tile([128, RT], f32, tag="pos")

            def load_group(g, qs=range(4)):
                """DMA 512-row chunks of group g."""
                for q in qs:
                    src = x[g * GCOLS + q * 512 : g * GCOLS + (q + 1) * 512, :]
                    src = src.rearrange("(s p) d -> p s d", p=128)
                    dst = xg[g][:, q * 512 : (q + 1) * 512].rearrange(
                        "p (s d) -> p s d", s=4
                    )
                    eng = nc.sync if q % 2 == 0 else nc.scalar
                    eng.dma_start(out=dst, in_=src)

            def norm_chunk(g, q, dve):
                """nsq -> rno for 512 rows: squares+reduce, then ACT rsqrt."""
                sl = slice(q * 512, (q + 1) * 512)
                eng = nc.vector if dve else nc.gpsimd
                eng.tensor_mul(sq[g][:, sl], xg[g][:, sl], xg[g][:, sl])
                nc.vector.tensor_reduce(
                    nsq[g][:, q * 4 : (q + 1) * 4],
                    sq[g][:, sl].rearrange("p (s d) -> p s d", s=4),
                    axis=AX.X, op=OP.add,
                )
                lnv = smallp.tile([128, 4], f32, tag="lnv")
                nc.scalar.activation(lnv[:], nsq[g][:, q * 4 : (q + 1) * 4],
                                     AF.Ln, bias=eps2[:, 0:1])
                nc.scalar.activation(rno[g][:, q * 4 : (q + 1) * 4],
                                     lnv[:], AF.Exp, scale=-0.5)

            def scale_chunk(g, q):
                """zbg = xg * rno (per 128-col block, f32 -> bf16)."""
                for j in range(4):
                    s = 4 * q + j
                    nc.vector.tensor_scalar_mul(
                        zbg[g][:, s * 128 : (s + 1) * 128],
                        xg[g][:, s * 128 : (s + 1) * 128],
                        rno[g][:, s : s + 1],
                    )

            def transpose_chunk(g, q, ptr):
                """PE-transpose 512 cols of zbg into psum, DVE-copy to zbT."""
                for j in range(4):
                    s = 4 * q + j
                    nc.tensor.transpose(
                        ptr[:, s * 128 : (s + 1) * 128],
                        zbg[g][:, s * 128 : (s + 1) * 128],
                        ident[:],
                    )
                nc.vector.tensor_copy(
                    zbT[g][:, q * 512 : (q + 1) * 512],
                    ptr[:, q * 512 : (q + 1) * 512],
                )

            def mm_chunk(g, m, sub_exp=False):
                pt = psump.tile([128, GCOLS], f32, tag="pt")
                lhs = zbT[0][:, m * 128 : (m + 1) * 128]
                for k in range(4):
                    nc.tensor.matmul(
                        pt[:, k * 512 : (k + 1) * 512],
                        lhs,
                        zbT[g][:, k * 512 : (k + 1) * 512],
                    )
                    if sub_exp:
                        nc.scalar.activation(
                            et[m % 2][:, k * 512 : (k + 1) * 512],
                            pt[:, k * 512 : (k + 1) * 512],
                            AF.Exp, scale=ESC,
                            accum_out=racc[:, G * RT + k : G * RT + k + 1],
                        )
                if sub_exp:
                    return
                if (g, m) in DVE_CHUNKS:
                    # Schraudolph: bits(exp(10*s)) = int32(s*A10 + BS)
                    nc.vector.tensor_scalar(
                        out=ei[m % 2][:],
                        in0=pt[:],
                        scalar1=A10,
                        scalar2=BS,
                        op0=OP.mult,
                        op1=OP.add,
                    )
                    nc.vector.tensor_reduce(
                        rdve[:, g * RT + m : g * RT + m + 1],
                        ei[m % 2][:].bitcast(f32),
                        axis=AX.X, op=OP.add,
                    )
                else:
                    nc.scalar.activation(
                        et[m % 2][:], pt[:], AF.Exp, scale=ESC,
                        accum_out=racc[:, g * RT + m : g * RT + m + 1],
                    )

            # ---- head: group 0 pipelined at 512-row granularity; the
            # (0, m=0) chunk is emitted as 4 512-col sub-chunks interleaved
            # with the per-512-row preamble so the first exp runs ~9us in.
            load_group(0)
            pt0 = psump.tile([128, GCOLS], f32, tag="pt")
            ptr0 = psump.tile([128, GCOLS], f32, tag="pt", name="ptr0").bitcast(bf16)[:, 0:GCOLS]
            lhs0 = [None]
            for q in range(4):
                norm_chunk(0, q, dve=True)
                scale_chunk(0, q)
                transpose_chunk(0, q, ptr0)
                if q == 0:
                    lhs0[0] = zbT[0][:, 0:128]
                nc.tensor.matmul(
                    pt0[:, q * 512 : (q + 1) * 512],
                    lhs0[0],
                    zbT[0][:, q * 512 : (q + 1) * 512],
                )
                nc.scalar.activation(
                    et[0][:, q * 512 : (q + 1) * 512],
                    pt0[:, q * 512 : (q + 1) * 512],
                    AF.Exp, scale=ESC,
                    accum_out=racc[:, G * RT + q : G * RT + q + 1],
                )
            load_group(1)

            # ---- main stream: g outer, m inner ----
            for g in range(G):
                for m in range(RT):
                    if g == 0 and m == 0:
                        continue  # emitted in the head
                    mm_chunk(g, m)
                    if g + 1 < G:
                        if m == 1:
                            for q in range(4):
                                norm_chunk(g + 1, q, dve=False)
                        elif m == 2:
                            scale_chunk(g + 1, 0)
                            scale_chunk(g + 1, 1)
                        elif m == 3:
                            scale_chunk(g + 1, 2)
                            scale_chunk(g + 1, 3)
                        elif m == 4:
                            ptr = psump.tile([128, GCOLS], f32, tag="pt", name=f"ptr{g}").bitcast(bf16)[:, 0:GCOLS]
                            for q in range(4):
                                transpose_chunk(g + 1, q, ptr)
                        elif m == 5 and g + 2 < G:
                            load_group(g + 2)
                if g == 2:
                    # positives: pos[p, s] = sum_d zbg0[p,s,d] * zbg2[p,s,d]
                    pz = bigp.tile([128, RPC], bf16, tag="pz")
                    nc.vector.tensor_mul(pz[:], zbg[0][:, 0:RPC], zbg[2][:, 0:RPC])
                    nc.vector.tensor_reduce(
                        pos[:],
                        pz[:].rearrange("p (s d) -> p s d", s=RT),
                        axis=AX.X, op=OP.add,
                    )

            # ---- epilogue: loss = ln(rowsum - e^10) - 10*pos ----
            tot = smallp.tile([128, RT], f32, tag="tot")
            nc.vector.tensor_reduce(
                tot[:],
                racc[:, 0 : G * RT].rearrange("p (g m) -> p m g", m=RT),
                axis=AX.X, op=OP.add,
            )
            totd = smallp.tile([128, RT], f32, tag="totd")
            nc.vector.tensor_reduce(
                totd[:],
                rdve[:].rearrange("p (g m) -> p m g", m=RT),
                axis=AX.X, op=OP.add,
            )
            th = smallp.tile([128, 1], f32, tag="th")
            nc.vector.tensor_reduce(
                th[:], racc[:, G * RT : G * RT + 4], axis=AX.X, op=OP.add
            )
            # fold the head sub-chunk sums into m=0; add ACT+DVE partials
            nc.vector.tensor_add(tot[:, 0:1], tot[:, 0:1], th[:])
            nc.vector.tensor_add(tot[:], tot[:], totd[:])
            ndall = smallp.tile([128, RT], f32, tag="ndall")
            nc.vector.tensor_scalar_add(ndall[:], tot[:], -E10)
            lnd = smallp.tile([128, RT], f32, tag="lnd")
            nc.scalar.activation(lnd[:], ndall[:], AF.Ln)
            lt = smallp.tile([128, RT], f32, tag="lt")
            nc.vector.scalar_tensor_tensor(
                out=lt[:], in0=pos[:], scalar=-ESC, in1=lnd[:],
                op0=OP.mult, op1=OP.add,
            )
            nc.sync.dma_start(out=out, in_=lt[:])

    nc.compile()
    return nc


def _get_nc():
    if "nc" not in _CACHE:
        _CACHE["nc"] = _build()
    return _CACHE["nc"]


def kernel(stacked_batch: np.ndarray) -> np.ndarray:
    global LAST_RESULTS
    from concourse.bass_utils import run_bass_kernel_spmd

    nc = _get_nc()
    xf = np.ascontiguousarray(np.asarray(stacked_batch, dtype=np.float32))
    assert xf.shape == (N, D)

    ident = np.eye(128, dtype=np.float32)
    in_maps = [
        {"x": np.ascontiguousarray(np.roll(xf, -c * RPC, axis=0)), "ident": ident}
        for c in range(NCORES)
    ]
    res = run_bass_kernel_spmd(
        nc,
        in_maps,
        core_ids=list(range(NCORES)),
        trace=bool(os.environ.get("BASS_TRACE")),
    )
    LAST_RESULTS = res
    total = 0.0
    for c in range(NCORES):
        total += float(np.asarray(res.results[c]["loss_parts"], dtype=np.float64).sum())
    return np.float32(total / N)
